# revision 15
# baseline (speedup 1.0000x reference)
"""Trainium2 Bass kernel for nn_CAttention (channel attention).

Reference computation (per batch b):
    k      = einsum('cit,i->ct', x[b], alpha)          # [C, T]
    scores = k @ W @ k.T                               # [C, C]
    att    = softmax(scores, axis=-1)
    out[b] = att @ x[b].reshape(C, N*T)                # [C, N*T]

Shapes (hardcoded): x [64, 256, 307, 12], W [12, 12], alpha [307].
Sharding: data-parallel over batch B across 8 cores (8 batches/core);
W and alpha replicated.

The kernel is HBM-DMA bound, so x and out travel as fp16 (host casts
f32->fp16 in, fp16->f32 back): 15.1 MB in + 15.1 MB out per core =>
~85 us floor at the ~350 GB/s effective per-core DMA rate.  fp16
everywhere measures ~1.5e-3 L2 end-to-end (bf16 weights measured
1.3e-2 on HW - too close to the gate; and NOTE: a mixed bf16 x fp16
matmul FAULTS the device, NRT_EXEC_UNIT_UNRECOVERABLE, so every
matmul here keeps both operands the same dtype).

exp(scores) reaches e^31 which overflows fp16, so attention weights
are normalized BEFORE they become matmul weights: scores are computed
in row orientation [c-part, d-free] (stationary kWT chunk x moving
kT), exp on ACT emits f32 weights plus the softmax denominator via
accum_out in the same pass, DVE takes one reciprocal, ACT's normalize
multiply (per-partition rinv) emits fp16 weights <= 1, and four PE
transposes flip them to the [d-part, c] stationary layout.

Per-engine budget per batch (target period ~10.5 us):
 - DMA: one 1.9 MB batch load (SP ring) + two 0.95 MB stores (ACT).
 - DVE ~9.9: owns k-pooling: packed-2x fp16 multiply by a materialized
   alpha-per-(i,t) row, in-place packed fold tree 307->154->77->39->20
   i's, one strided reduce_sum.  (TensorReduce/TensorTensorReduce have
   no DVE fast modes, and GpSimd tensor ops run at 0.42 efficiency and
   contend for DVE's tensor_tensor port, so all-DVE packed is optimal;
   the Pool engine stays idle on purpose.)  Plus the one reciprocal.
 - ACT ~9.9: PSUM drains as 3 copies per c-chunk (1536/1536/612 cols
   spanning 3/3/2 banks - fewer, larger copies; ACT gets no 16-bit
   speedup so element count and instruction count are what matter),
   kT/kWT evacuations (casting to fp16), exp+accum, normalize.
 - PE ~7.5: big matmul all-fp16 at 1 col/cyc, fp16 scores matmuls.
 - PSUM: two 3-bank tiles (big-matmul groups, ping-pong) + two 1-bank
   tiles (scores chain) = exactly 8 banks, no pool conflicts.

Three-stage pipeline with one-step load prefetch; the emission order
interleaves engines so every cross-engine hop has queued work and the
DVE order is pool(c0), pool(c1), recip.
"""

from contextlib import ExitStack

import numpy as np

import concourse.bass as bass
import concourse.bass_utils as _bass_utils
import concourse.tile as tile
from concourse import bacc, mybir
from concourse.bass import ts
from concourse.bass_utils import run_bass_kernel_spmd
from concourse.masks import make_identity

B, C, N, T = 64, 256, 307, 12
NCORES = 8
B_LOC = B // NCORES          # 8 batches per core
F = N * T                    # 3684 flattened free dim
P = 128                      # partitions
CC = C // P                  # 2 c-chunks

# Big-matmul PSUM groups per c-chunk: (f0, size, [sub-tile sizes]).
# Each sub-tile is one matmul dest (<=512 cols, bank-aligned inside a
# 3-bank group tile); each group drains with a single ACT copy.
_GROUPS = [
    (0, 1024, (512, 512)),
    (1024, 1024, (512, 512)),
    (2048, 1024, (512, 512)),
    (3072, 612, (512, 100)),
]

# In-place fold tree for the i-reduction, in element offsets (i*T).
# Each level folds src range [s0, s1) onto dst [d0, d0+(s1-s0)); all
# ranges are 4B-aligned with even element counts so fp16 tensor_add
# runs in packed 2x_1p mode.  Afterwards i in [0, 20) remains.
_FOLDS = [
    (12, 1848, 3684),   # i[154..307) -> i[1..154)
    (0, 924, 1848),     # i[77..154)  -> i[0..77)
    (12, 468, 924),     # i[39..77)   -> i[1..39)
    (12, 240, 468),     # i[20..39)   -> i[1..20)
]
_REM = 20                    # i's remaining for the final reduce_sum

_F32 = mybir.dt.float32
_F16 = mybir.dt.float16


def _emit_core_kernel(tc, x_ap, w_ap, alpha_ap, out_ap):
    """Emit the per-core program. x_ap/out_ap: [B_LOC, C, N, T] DRAM fp16."""
    nc = tc.nc
    ctx = ExitStack()

    x_flat = x_ap.rearrange("b c i t -> b c (i t)")      # [B_LOC, C, F]
    out_flat = out_ap.rearrange("b c i t -> b c (i t)")  # [B_LOC, C, F]

    consts = ctx.enter_context(tc.tile_pool(name="consts", bufs=1))
    xpool = ctx.enter_context(tc.tile_pool(name="x", bufs=7))
    prodpool = ctx.enter_context(tc.tile_pool(name="prod", bufs=2))
    kpool = ctx.enter_context(tc.tile_pool(name="k", bufs=3))
    ktpool = ctx.enter_context(tc.tile_pool(name="kt", bufs=3))
    apool = ctx.enter_context(tc.tile_pool(name="att32", bufs=2))
    attpool = ctx.enter_context(tc.tile_pool(name="att", bufs=4))
    outpool = ctx.enter_context(tc.tile_pool(name="out", bufs=2))
    rpool = ctx.enter_context(tc.tile_pool(name="rinv", bufs=3))
    # PSUM: 2 x 3-bank big-matmul group tiles + 2 x 1-bank scores tiles
    psA = ctx.enter_context(tc.tile_pool(name="psA", bufs=2, space="PSUM"))
    psS = ctx.enter_context(tc.tile_pool(name="psS", bufs=4, space="PSUM"))

    # Constants: identity for PE transposes, W (fp16 for same-dtype
    # matmuls), and the alpha row expanded to one fp16 weight per
    # (i, t) column so the pooling multiply is unit-stride packed.
    ident = consts.tile([P, P], _F32)
    make_identity(nc, ident)
    ident16 = consts.tile([P, P], _F16)
    make_identity(nc, ident16)
    w_sb = consts.tile([T, T], _F32)
    nc.gpsimd.dma_start(out=w_sb, in_=w_ap)
    w16 = consts.tile([T, T], _F16)
    nc.vector.tensor_copy(w16, w_sb)
    alpha_row = consts.tile([P, N], _F32)
    nc.gpsimd.dma_start(out=alpha_row, in_=alpha_ap[None, :].to_broadcast([P, N]))
    alpha_full = consts.tile([P, F], _F16)
    nc.vector.tensor_copy(
        alpha_full.rearrange("p (i t) -> p i t", t=T),
        alpha_row[:, :, None].to_broadcast([P, N, T]),
    )

    def phase_load(b):
        """One DMA for the whole batch (1.9 MB, SP ring)."""
        x_t = xpool.tile([P, CC, F], _F16, tag="x")
        nc.sync.dma_start(
            out=x_t, in_=x_flat[b].rearrange("(cc p) f -> p cc f", p=P)
        )
        k_c = kpool.tile([P, CC, T], _F32, tag="k")
        return {"x_t": x_t, "k_c": k_c}

    def phase_pool(st, cc):
        """k for one c-chunk, entirely on DVE in packed 2x mode."""
        prod = prodpool.tile([P, F], _F16, tag="prod")
        nc.vector.tensor_mul(prod, st["x_t"][:, cc, :], alpha_full)
        for d0, s0, s1 in _FOLDS:
            n = s1 - s0
            nc.vector.tensor_add(
                prod[:, d0 : d0 + n], prod[:, d0 : d0 + n], prod[:, s0:s1]
            )
        nc.vector.reduce_sum(
            out=st["k_c"][:, cc, :],
            in_=prod[:, : _REM * T].rearrange("p (i t) -> p t i", t=T),
            axis=mybir.AxisListType.X,
        )

    def phase_kt_a(st):
        """Transpose k chunk 0 -> kT (PE; can start mid-pool)."""
        ps_kt = psS.tile([P, 512], _F32, tag="ps", name="ps_kt")
        nc.tensor.transpose(ps_kt[:T, 0:P], st["k_c"][:, 0, :], ident)
        st["ps_kt"] = ps_kt

    def phase_kt_b(st):
        """Transpose k chunk 1, evacuate kT as fp16."""
        ps_kt = st.pop("ps_kt")
        nc.tensor.transpose(ps_kt[:T, P:C], st["k_c"][:, 1, :], ident)
        kt16 = ktpool.tile([T, C], _F16, tag="kt")
        nc.scalar.copy(out=kt16, in_=ps_kt[:T, :C])
        st["kt16"] = kt16

    def phase_scores_b(st):
        """kWT = W^T kT; scores rows; exp (+denominator via accum)."""
        kt16 = st["kt16"]
        ps_kwt = psS.tile([P, 512], _F32, tag="ps", name="ps_kwt")
        nc.tensor.matmul(
            ps_kwt[:T, :C], lhsT=w16, rhs=kt16, start=True, stop=True
        )
        kwt16 = ktpool.tile([T, C], _F16, tag="kwt")
        nc.scalar.copy(out=kwt16, in_=ps_kwt[:T, :C])
        # scores[c, d] rows: c of this chunk on partitions, d free
        ps_sc = psS.tile([P, 512], _F32, tag="ps", name="ps_sc")
        for cc in range(CC):
            nc.tensor.matmul(
                ps_sc[:, ts(cc, C)],
                lhsT=kwt16[:, ts(cc, P)],
                rhs=kt16,
                start=True,
                stop=True,
            )
        att32 = apool.tile([P, CC, C], _F32, tag="att32")
        den = rpool.tile([P, CC, 2], _F32, tag="den")
        for cc in range(CC):
            nc.scalar.activation(
                out=att32[:, cc, :],
                in_=ps_sc[:, ts(cc, C)],
                func=mybir.ActivationFunctionType.Exp,
                accum_out=den[:, cc, 0:1],
            )
        st["att32"] = att32
        st["den"] = den

    def phase_recip(st):
        """DVE reciprocal of the softmax denominators (both chunks)."""
        nc.vector.reciprocal(out=st["den"][:, :, 1], in_=st["den"][:, :, 0])

    def phase_att_fin(st):
        """Normalize to fp16 weights and transpose to [d-part, c]."""
        att32, den = st["att32"], st["den"]
        a16r = apool.tile([P, CC, C], _F16, tag="a16r")
        for cc in range(CC):
            nc.scalar.mul(
                out=a16r[:, cc, :], in_=att32[:, cc, :], mul=den[:, cc, 1:2]
            )
        ps_at = psS.tile([P, 512], _F16, tag="ps", name="ps_at")
        for dc in range(CC):
            for cc in range(CC):
                nc.tensor.transpose(
                    ps_at[:, ts(dc * CC + cc, P)],
                    a16r[:, cc, ts(dc, P)],
                    ident16,
                )
        att_t = attpool.tile([P, CC, C], _F16, tag="attT")
        nc.scalar.copy(out=att_t.rearrange("p a c -> p (a c)"), in_=ps_at)
        st["att_t"] = att_t

    def phase_mm_fin(b, st, cc):
        """Big matmul (all fp16) + drains for one c-chunk, grouped in
        3 PSUM group-tiles with one ACT drain copy each."""
        x_t, att_t = st["x_t"], st["att_t"]
        if cc == 0:
            st["o_t"] = outpool.tile([P, CC, F], _F16, tag="o", name="o_t")
        o_t = st["o_t"]
        for gi, (g0, gsz, subs) in enumerate(_GROUPS):
            pt = psA.tile([P, 1024], _F32, tag="ps_mm", name=f"ps_mm{gi}")
            for dc in range(CC):
                s0 = 0
                for ssz in subs:
                    nc.tensor.matmul(
                        pt[:, s0 : s0 + ssz],
                        lhsT=att_t[:, dc, ts(cc, P)],
                        rhs=x_t[:, dc, g0 + s0 : g0 + s0 + ssz],
                        start=(dc == 0),
                        stop=(dc == CC - 1),
                    )
                    s0 += ssz
            nc.scalar.copy(out=o_t[:, cc, g0 : g0 + gsz], in_=pt[:, :gsz])
        nc.scalar.dma_start(out=out_flat[b, ts(cc, P), :], in_=o_t[:, cc, :])

    # Four-stage pipeline (load prefetched two steps ahead): step s
    # runs the big matmul for batch s-3, the scores chain for batch
    # s-1 (whose k was pooled last step), pooling for batch s, and the
    # normalize/transpose tail for s-1 at the end - so attention
    # weights are final one full period before mm needs them and the
    # serial exp->recip->normalize->transpose tail never stalls the PE
    # stream (which also keeps the HAM clock gate warm).
    states = {}
    for s in range(-2, B_LOC + 3):
        if 0 <= s + 2 < B_LOC:
            states[s + 2] = phase_load(s + 2)
        st3 = states.get(s - 3)
        st1 = states.get(s - 1)
        st0 = states.get(s)
        if st3 is not None:
            phase_mm_fin(s - 3, st3, 0)
        if st1 is not None:
            phase_kt_a(st1)
            phase_kt_b(st1)
        if st0 is not None:
            phase_pool(st0, 0)
        if st3 is not None:
            phase_mm_fin(s - 3, st3, 1)
            states.pop(s - 3)
        if st1 is not None:
            phase_scores_b(st1)
        if st0 is not None:
            phase_pool(st0, 1)
        if st1 is not None:
            phase_recip(st1)
            phase_att_fin(st1)
    ctx.close()


_CACHED_NC = None


def _build():
    global _CACHED_NC
    if _CACHED_NC is not None:
        return _CACHED_NC
    nc = bacc.Bacc("TRN2", target_bir_lowering=False, debug=False, num_devices=NCORES)
    x_d = nc.dram_tensor("x", [B_LOC, C, N, T], _F16, kind="ExternalInput").ap()
    w_d = nc.dram_tensor("W", [T, T], _F32, kind="ExternalInput").ap()
    a_d = nc.dram_tensor("alpha", [N], _F32, kind="ExternalInput").ap()
    o_d = nc.dram_tensor("out", [B_LOC, C, N, T], _F16, kind="ExternalOutput").ap()
    with tile.TileContext(nc) as tc:
        _emit_core_kernel(tc, x_d, w_d, a_d, o_d)
    nc.compile()
    _CACHED_NC = nc
    return nc


def run(x, W, alpha, trace=False, **spmd_kwargs):
    """Run on 8 cores; returns (full output [B,C,N,T], BassKernelResults)."""
    x = np.ascontiguousarray(np.asarray(x, dtype=np.float32))
    W = np.ascontiguousarray(np.asarray(W, dtype=np.float32))
    alpha = np.ascontiguousarray(np.asarray(alpha, dtype=np.float32))
    assert x.shape == (B, C, N, T) and W.shape == (T, T) and alpha.shape == (N,)

    x16 = x.astype(np.float16)
    nc = _build()
    in_maps = [
        {"x": x16[i * B_LOC : (i + 1) * B_LOC], "W": W, "alpha": alpha}
        for i in range(NCORES)
    ]
    res = run_bass_kernel_spmd(
        nc, in_maps, core_ids=list(range(NCORES)), trace=trace, **spmd_kwargs
    )
    out = np.concatenate([r["out"] for r in res.results], axis=0).astype(np.float32)
    return out, res


def kernel(x, W, alpha):
    out, _ = run(x, W, alpha)
    return out
